# revision 14
# baseline (speedup 1.0000x reference)
"""AGD loss (angular-Gaussian density contrastive loss) on 8 TRN2 NeuronCores.

Math: the reference evaluates, per column j (n = V*B = 32768 columns) and per
class c (C = 100), the Saw-series density s(y[c,j]) where
    s(a) = sum_n c_n a^n,   c_n = 2^{n/2} Gamma((d+n)/2) / (Gamma(d/2) n!)
(the huge exp(log_Cd - 1/(2 sigma^2)) prefactor cancels in the final
log-ratio).  The coefficients c_n * n! are exactly the raw moments of a
chi(d=128) random variable R, so

    s(a) = E_R[exp(R * a)]  (40-term truncation is far below fp32 noise)

and an M-point Gauss quadrature on the chi(128) weight collapses the 40-term
polynomial to M=4 exponentials:

    s(a) ~= sum_m exp(r_m * a + ln w_m)      (max rel err ~3e-7 on |a|<=0.55)

On-device per core (data-parallel over columns, 4096 columns/core):
  - input tile X [101, 4096] fp16: rows 0..99 = y classes, row 100 = the
    host-gathered own-class value y[label_j, j]
  - ScalarE: M activation(Exp, scale=r_m, bias=ln w_m) passes (fp16 out)
  - TensorE: [101->2] ones/one-hot matmul accumulating the M terms in PSUM:
    row 0 = norms_j = sum_c s(y[c,j]),  row 1 = s_lab_j
  - ScalarE: Ln over PSUM with accum_out -> per-chunk partial sums [2,1]
  - host: loss = sum(log norms) - sum(log s_lab), summed in float64
"""

import numpy as np

import concourse.bass as bass
import concourse.bacc as bacc
import concourse.mybir as mybir
from concourse.tile import TileContext
from concourse.bass_utils import run_bass_kernel_spmd

N_CORES = 8
B = 16384
V = 2
D = 128
C = 100
N = V * B                 # 32768 columns
NLOC = N // N_CORES       # 4096 columns per core
P = 128                   # 100 class rows + 1 own-class row + 27 zero pad rows
FC = 2048                 # columns per chunk
NCHUNK = NLOC // FC
MM_N = 512                # matmul moving free dim
NACC = NLOC // MM_N       # one Ln + accum slot per PSUM bank

# Equispaced-node exponential fit of the chi(128) MGF on |a| <= 0.6:
#   s(a) ~= e^{R0 a} * (W0 + W1 B + W2 B^2 + W3 B^3 + W4 B^4),  B = e^{DLT a}
# (max rel err 4.9e-7).  Only 2 ScalarE exps; the Horner chain in B runs on
# VectorE as 1 tensor_scalar + 4 scalar_tensor_tensor fp16 ops.
R0 = 9.410000
DLT = 0.962500
W = [0.014169619256263121, 0.2366923174335881, 0.5428259482952243,
     0.1926491984255905, 0.013663071054319072]

IN_DT = mybir.dt.float16

_CACHE = {}
LAST_RESULT = None  # BassKernelResults of the most recent run (for profiling)
TRACE = False


def _patch_act_tables():
    """Make exp and ln both resolve to natural_log_exp_and_others so the
    kernel needs a single ACT table load instead of two (~2.7us each).
    Indices into act_info.json must be preserved, so we blank out the
    competing sets rather than reordering."""
    import concourse.hw_specs as hw_specs

    if getattr(hw_specs, "_agd_patched", False):
        return
    orig = hw_specs.get_activation_tables

    def patched(module_arch):
        tabs = orig(module_arch)
        exp_f = mybir.ActivationFunctionType.Exp
        ln_f = mybir.ActivationFunctionType.Ln
        out = {}
        for name, funcs in tabs.items():
            if name != "natural_log_exp_and_others" and (exp_f in funcs or ln_f in funcs):
                funcs = funcs - {exp_f, ln_f}
            out[name] = funcs
        return out

    hw_specs.get_activation_tables = patched
    bacc.get_activation_tables = patched
    hw_specs._agd_patched = True


def build_bass():
    _patch_act_tables()
    nc = bacc.Bacc(None, target_bir_lowering=False)
    x = nc.declare_dram_parameter("x", [NCHUNK, P, FC], IN_DT, isOutput=False)
    sel_in = nc.declare_dram_parameter("sel", [P, 2], IN_DT, isOutput=False)
    out = nc.declare_dram_parameter("out", [2, NCHUNK], mybir.dt.float32, isOutput=True)

    with TileContext(nc) as tc:
        with (
            tc.tile_pool(name="const", bufs=1) as cpool,
            tc.tile_pool(name="xin", bufs=2) as xpool,
            tc.tile_pool(name="exp", bufs=3) as epool,
            tc.tile_pool(name="ln", bufs=1) as lpool,
            tc.tile_pool(name="acc", bufs=1) as apool,
            tc.tile_pool(name="ps", bufs=1, space="PSUM") as ppool,
        ):
            # selection matrix: col 0 sums the 100 class rows (norms),
            # col 1 picks row 100 (own-class density)
            sel = cpool.tile([P, 2], IN_DT)
            nc.sync.dma_start(sel[:, :], sel_in[:, :])

            # tiny warm-up Exp: triggers the (single, patched) ACT table
            # load while the input DMA is in flight
            warm = cpool.tile([2, 2], mybir.dt.float32)
            nc.vector.memset(warm[:, 1:2], 0.0)
            nc.scalar.activation(
                warm[:, 0:1], warm[:, 1:2], mybir.ActivationFunctionType.Exp
            )

            acc = apool.tile([2, NCHUNK], mybir.dt.float32)
            ps = ppool.tile([2, NLOC], mybir.dt.float32)

            lt = lpool.tile([2, NLOC], mybir.dt.float32)
            xts = []
            for k in range(NCHUNK):
                xt = xpool.tile([P, FC], IN_DT)
                nc.gpsimd.dma_start(xt[:, :], x[k, :, :])
                xts.append(xt)
            add_ = mybir.AluOpType.add
            mult_ = mybir.AluOpType.mult
            for k in range(NCHUNK):
                bt = epool.tile([P, FC], IN_DT, tag="bt")
                nc.scalar.activation(
                    bt[:, :], xts[k][:, :], mybir.ActivationFunctionType.Exp,
                    scale=DLT,
                )
                at = epool.tile([P, FC], IN_DT, tag="at")
                nc.scalar.activation(
                    at[:, :], xts[k][:, :], mybir.ActivationFunctionType.Exp,
                    scale=R0,
                )
                v0 = epool.tile([P, FC], IN_DT, tag="v0")
                nc.vector.tensor_scalar_mul(v0[:, :], bt[:, :], float(W[4]))
                v1 = epool.tile([P, FC], IN_DT, tag="v1")
                nc.vector.scalar_tensor_tensor(
                    v1[:, :], v0[:, :], float(W[3]), bt[:, :], op0=add_, op1=mult_
                )
                v2 = epool.tile([P, FC], IN_DT, tag="v0")
                nc.vector.scalar_tensor_tensor(
                    v2[:, :], v1[:, :], float(W[2]), bt[:, :], op0=add_, op1=mult_
                )
                v3 = epool.tile([P, FC], IN_DT, tag="v1")
                nc.vector.scalar_tensor_tensor(
                    v3[:, :], v2[:, :], float(W[1]), bt[:, :], op0=add_, op1=mult_
                )
                st = epool.tile([P, FC], IN_DT, tag="st")
                nc.vector.scalar_tensor_tensor(
                    st[:, :], v3[:, :], float(W[0]), at[:, :], op0=add_, op1=mult_
                )
                for b in range(FC // MM_N):
                    col = k * FC + b * MM_N
                    nc.tensor.matmul(
                        ps[:, col : col + MM_N],
                        sel[:, :],
                        st[:, b * MM_N : (b + 1) * MM_N],
                        start=True,
                        stop=True,
                    )
                nc.scalar.activation(
                    lt[:, k * FC : (k + 1) * FC],
                    ps[:, k * FC : (k + 1) * FC],
                    mybir.ActivationFunctionType.Ln,
                    accum_out=acc[:, k : k + 1],
                )
            nc.sync.dma_start(out[:, :], acc[:, :])

    nc.finalize()
    return nc


def _get_nc():
    if "nc" not in _CACHE:
        _CACHE["nc"] = build_bass()
    return _CACHE["nc"]


def kernel(features: np.ndarray, labels: np.ndarray) -> np.ndarray:
    global LAST_RESULT
    features = np.asarray(features)
    labels = np.asarray(labels)

    # view-major flatten: [B, V, D] -> [V*B, D]
    feats = np.ascontiguousarray(features.transpose(1, 0, 2).reshape(N, D))
    labels_rep = np.tile(labels.astype(np.int64), V)
    alab = feats[np.arange(N), labels_rep]  # own-class coordinate per column

    sel_np = np.zeros((P, 2), dtype=np.float16)
    sel_np[:C, 0] = 1.0
    sel_np[C, 1] = 1.0

    in_maps = []
    for i in range(N_CORES):
        sl = slice(i * NLOC, (i + 1) * NLOC)
        X = np.zeros((P, NLOC), dtype=np.float16)
        X[:C, :] = feats[sl, :C].T
        X[C, :] = alab[sl]
        X3 = np.ascontiguousarray(X.reshape(P, NCHUNK, FC).transpose(1, 0, 2))
        in_maps.append({"x": X3, "sel": sel_np})

    nc = _get_nc()
    res = run_bass_kernel_spmd(nc, in_maps, list(range(N_CORES)), trace=TRACE)
    LAST_RESULT = res

    total = np.float64(0.0)
    for i in range(N_CORES):
        o = res.results[i]["out"].astype(np.float64)
        total += o[0].sum() - o[1].sum()
    return np.asarray(total, dtype=np.float64)


# revision 15
# speedup vs baseline: 1.4495x; 1.4495x over previous
"""AGD loss (angular-Gaussian density contrastive loss) on 8 TRN2 NeuronCores.

Math: the reference evaluates, per column j (n = V*B = 32768 columns) and per
class c (C = 100), the Saw-series density s(y[c,j]) where
    s(a) = sum_n c_n a^n,   c_n = 2^{n/2} Gamma((d+n)/2) / (Gamma(d/2) n!)
(the huge exp(log_Cd - 1/(2 sigma^2)) prefactor cancels in the final
log-ratio).  The coefficients c_n * n! are exactly the raw moments of a
chi(d=128) random variable R, so

    s(a) = E_R[exp(R * a)]  (40-term truncation is far below fp32 noise)

and an M-point Gauss quadrature on the chi(128) weight collapses the 40-term
polynomial to M=4 exponentials:

    s(a) ~= sum_m exp(r_m * a + ln w_m)      (max rel err ~3e-7 on |a|<=0.55)

On-device per core (data-parallel over columns, 4096 columns/core):
  - input tile X [101, 4096] fp16: rows 0..99 = y classes, row 100 = the
    host-gathered own-class value y[label_j, j]
  - ScalarE: M activation(Exp, scale=r_m, bias=ln w_m) passes (fp16 out)
  - TensorE: [101->2] ones/one-hot matmul accumulating the M terms in PSUM:
    row 0 = norms_j = sum_c s(y[c,j]),  row 1 = s_lab_j
  - ScalarE: Ln over PSUM with accum_out -> per-chunk partial sums [2,1]
  - host: loss = sum(log norms) - sum(log s_lab), summed in float64
"""

import numpy as np

import concourse.bass as bass
import concourse.bacc as bacc
import concourse.mybir as mybir
from concourse.tile import TileContext
from concourse.bass_utils import run_bass_kernel_spmd

N_CORES = 8
B = 16384
V = 2
D = 128
C = 100
N = V * B                 # 32768 columns
NLOC = N // N_CORES       # 4096 columns per core
P = 128                   # 100 class rows + 1 own-class row + 27 zero pad rows
FC = 2048                 # columns per chunk
NCHUNK = NLOC // FC
MM_N = 512                # matmul moving free dim
NACC = NLOC // MM_N       # one Ln + accum slot per PSUM bank

# Equispaced-node exponential fit of the chi(128) MGF on |a| <= 0.6 with the
# leading weight folded into the B exponent (max rel err 3.0e-6, final 2.8e-7):
#   s(a) ~= A * (B'^3 + W2P B'^2 + W1P B' + W0)
#   B' = e^{DLT a + B0},  A = e^{R0 a}
# Only 2 ScalarE exps; the cubic runs on VectorE as 3x(tensor_scalar add @4x
# + tensor_tensor mult @2x) in fp16.
R0 = 8.7
DLT = 1.2775
B0 = -0.5926865923017232
W2P = 2.2554957611955113
W1P = 0.2588032927308701
W0 = -0.0013924175175444766

IN_DT = mybir.dt.float16

_CACHE = {}
LAST_RESULT = None  # BassKernelResults of the most recent run (for profiling)
TRACE = False


def _patch_act_tables():
    """Make exp and ln both resolve to natural_log_exp_and_others so the
    kernel needs a single ACT table load instead of two (~2.7us each).
    Indices into act_info.json must be preserved, so we blank out the
    competing sets rather than reordering."""
    import concourse.hw_specs as hw_specs

    if getattr(hw_specs, "_agd_patched", False):
        return
    orig = hw_specs.get_activation_tables

    def patched(module_arch):
        tabs = orig(module_arch)
        exp_f = mybir.ActivationFunctionType.Exp
        ln_f = mybir.ActivationFunctionType.Ln
        out = {}
        for name, funcs in tabs.items():
            if name != "natural_log_exp_and_others" and (exp_f in funcs or ln_f in funcs):
                funcs = funcs - {exp_f, ln_f}
            out[name] = funcs
        return out

    hw_specs.get_activation_tables = patched
    bacc.get_activation_tables = patched
    hw_specs._agd_patched = True


def build_bass():
    _patch_act_tables()
    nc = bacc.Bacc(None, target_bir_lowering=False)
    x = nc.declare_dram_parameter("x", [NCHUNK, P, FC], IN_DT, isOutput=False)
    sel_in = nc.declare_dram_parameter("sel", [P, 2], IN_DT, isOutput=False)
    out = nc.declare_dram_parameter("out", [2, NCHUNK], mybir.dt.float32, isOutput=True)

    with TileContext(nc) as tc:
        with (
            tc.tile_pool(name="const", bufs=1) as cpool,
            tc.tile_pool(name="xin", bufs=2) as xpool,
            tc.tile_pool(name="exp", bufs=3) as epool,
            tc.tile_pool(name="ln", bufs=1) as lpool,
            tc.tile_pool(name="acc", bufs=1) as apool,
            tc.tile_pool(name="ps", bufs=1, space="PSUM") as ppool,
        ):
            # selection matrix: col 0 sums the 100 class rows (norms),
            # col 1 picks row 100 (own-class density)
            sel = cpool.tile([P, 2], IN_DT)
            nc.sync.dma_start(sel[:, :], sel_in[:, :])

            # per-partition bias for the B' exp
            b0t = cpool.tile([P, 1], mybir.dt.float32)
            nc.vector.memset(b0t[:, :], B0)

            # tiny warm-up Exp: triggers the (single, patched) ACT table
            # load while the input DMA is in flight
            warm = cpool.tile([2, 2], mybir.dt.float32)
            nc.vector.memset(warm[:, 1:2], 0.0)
            nc.scalar.activation(
                warm[:, 0:1], warm[:, 1:2], mybir.ActivationFunctionType.Exp
            )

            acc = apool.tile([2, NCHUNK], mybir.dt.float32)
            ps = ppool.tile([2, NLOC], mybir.dt.float32)

            lt = lpool.tile([2, NLOC], mybir.dt.float32)
            xts = []
            for k in range(NCHUNK):
                xt = xpool.tile([P, FC], IN_DT)
                nc.gpsimd.dma_start(xt[:, :], x[k, :, :])
                xts.append(xt)
            for k in range(NCHUNK):
                bt = epool.tile([P, FC], IN_DT, tag="bt")
                nc.scalar.activation(
                    bt[:, :], xts[k][:, :], mybir.ActivationFunctionType.Exp,
                    scale=DLT, bias=b0t[:, :],
                )
                at = epool.tile([P, FC], IN_DT, tag="at")
                nc.scalar.activation(
                    at[:, :], xts[k][:, :], mybir.ActivationFunctionType.Exp,
                    scale=R0,
                )
                t0 = epool.tile([P, FC], IN_DT, tag="t0")
                nc.vector.tensor_scalar_add(t0[:, :], bt[:, :], W2P)
                u1 = epool.tile([P, FC], IN_DT, tag="u1")
                nc.vector.tensor_mul(u1[:, :], t0[:, :], bt[:, :])
                t1 = epool.tile([P, FC], IN_DT, tag="t0")
                nc.vector.tensor_scalar_add(t1[:, :], u1[:, :], W1P)
                u2 = epool.tile([P, FC], IN_DT, tag="u1")
                nc.vector.tensor_mul(u2[:, :], t1[:, :], bt[:, :])
                t2 = epool.tile([P, FC], IN_DT, tag="t0")
                nc.vector.tensor_scalar_add(t2[:, :], u2[:, :], W0)
                st = epool.tile([P, FC], IN_DT, tag="st")
                nc.vector.tensor_mul(st[:, :], t2[:, :], at[:, :])
                for b in range(FC // MM_N):
                    col = k * FC + b * MM_N
                    nc.tensor.matmul(
                        ps[:, col : col + MM_N],
                        sel[:, :],
                        st[:, b * MM_N : (b + 1) * MM_N],
                        start=True,
                        stop=True,
                    )
                nc.scalar.activation(
                    lt[:, k * FC : (k + 1) * FC],
                    ps[:, k * FC : (k + 1) * FC],
                    mybir.ActivationFunctionType.Ln,
                    accum_out=acc[:, k : k + 1],
                )
            nc.sync.dma_start(out[:, :], acc[:, :])

    nc.finalize()
    return nc


def _get_nc():
    if "nc" not in _CACHE:
        _CACHE["nc"] = build_bass()
    return _CACHE["nc"]


def kernel(features: np.ndarray, labels: np.ndarray) -> np.ndarray:
    global LAST_RESULT
    features = np.asarray(features)
    labels = np.asarray(labels)

    # view-major flatten: [B, V, D] -> [V*B, D]
    feats = np.ascontiguousarray(features.transpose(1, 0, 2).reshape(N, D))
    labels_rep = np.tile(labels.astype(np.int64), V)
    alab = feats[np.arange(N), labels_rep]  # own-class coordinate per column

    sel_np = np.zeros((P, 2), dtype=np.float16)
    sel_np[:C, 0] = 1.0
    sel_np[C, 1] = 1.0

    in_maps = []
    for i in range(N_CORES):
        sl = slice(i * NLOC, (i + 1) * NLOC)
        X = np.zeros((P, NLOC), dtype=np.float16)
        X[:C, :] = feats[sl, :C].T
        X[C, :] = alab[sl]
        X3 = np.ascontiguousarray(X.reshape(P, NCHUNK, FC).transpose(1, 0, 2))
        in_maps.append({"x": X3, "sel": sel_np})

    nc = _get_nc()
    res = run_bass_kernel_spmd(nc, in_maps, list(range(N_CORES)), trace=TRACE)
    LAST_RESULT = res

    total = np.float64(0.0)
    for i in range(N_CORES):
        o = res.results[i]["out"].astype(np.float64)
        total += o[0].sum() - o[1].sum()
    return np.asarray(total, dtype=np.float64)
